# revision 31
# baseline (speedup 1.0000x reference)
"""Trainium2 Bass kernel for nn_DeconvBlock (offset conv -> deformable conv
-> BN+SiLU -> ConvTranspose2d(4,2,1) -> BN+SiLU), data-parallel over batch
on 8 NeuronCores with sync-BN allreduces.

v3: phased schedule (prep -> deform -> AR1 -> convT -> AR2 -> out) with the
gather-token pipeline hoisted ahead of the bilinear-weight pipeline so the
gathers overlap prep of the next batch element; prep PSUM evacuations split
across DVE/ACT; host-side pre-transposed f16 weights; wide 3-op DVE combine
plus one GPSIMD add per tap; f16 I/O; one-time pad-border memsets.
"""
import contextlib
import sys

sys.path.insert(0, "/opt/trn_rl_repo")

import numpy as np

import concourse.bass as bass
import concourse.mybir as mybir
from concourse import masks
from concourse.bacc import Bacc
from concourse.bass_types import AP
from concourse.bass_utils import run_bass_kernel_spmd
from concourse.tile import TileContext
from concourse.vector_clock import ScopedClock

# ---------------------------------------------------------------------------
# This walrus build rejects >1 sync-wait command on a Drain: split the
# TileContext tail-drain waits into a chain of single-wait drains.
from concourse import tile as _tile


def _patched_drain_and_barrier(self, tick_clock, wait_clock):
    nc = self.nc
    gc = tick_clock.global_clock
    procs = [(None, proc, tick) for proc, tick in enumerate(gc) if tick > 0]
    for scope, proc, tick in procs:
        sc = ScopedClock()
        sc.require_at_least(scope, proc, tick)
        d = nc.sync.drain()
        wait_clock.add_sem_waits(d.ins, sc)
    if not procs:
        nc.sync.drain()
    nc.all_engine_barrier()
    assert self.sems is not None
    popped = nc._tile_sem_poison_stack.pop()
    assert popped is self._sem_poison
    nc.clear_and_free_semaphores(list(self.sems.allocated().values()))
    nc.all_engine_barrier()


_tile.TileContext._drain_and_barrier = _patched_drain_and_barrier
# ---------------------------------------------------------------------------

F32 = mybir.dt.float32
F16 = mybir.dt.float16
I16 = mybir.dt.int16
I32 = mybir.dt.int32
ALU = mybir.AluOpType
ACTF = mybir.ActivationFunctionType

B, C, CO, H, W = 16, 256, 256, 32, 32
P = H * W  # 1024
NCORES = 8
BPC = B // NCORES  # batch elems per core
HO, WO = 2 * H, 2 * W
PO = HO * WO  # 4096
EPS = 1e-5
PADW = 34  # padded image row stride (34x34)
PPIX = PADW * PADW  # 1156


def apv(base, off, dims, nparts=None):
    """Free-dim view of an SBUF AP: keep its partition dim (stride), replace
    free dims. dims entries are [step, count] in elements of the tile row."""
    p = [base.ap[0][0], nparts if nparts is not None else base.ap[0][1]]
    return AP(tensor=base.tensor, offset=base.offset + off, ap=[p] + [list(d) for d in dims])


def dview(base, off, dims):
    """Arbitrary flat view of a DRAM AP."""
    return AP(tensor=base.tensor, offset=base.offset + off, ap=[list(d) for d in dims])


def _consts():
    q = np.arange(128)
    j = np.arange(8)
    k = np.arange(9)
    ky = k // 3 - 1
    kx = k % 3 - 1
    h = 4 * j[None, :, None] + (q[:, None, None] // 32)
    w = (q % 32)[:, None, None] + np.zeros((1, 8, 1))
    hky = (h + ky[None, None, :]) + np.zeros((128, 8, 9))
    wkx = (w + kx[None, None, :]) + np.zeros((128, 8, 9))
    rep16 = np.zeros((16, 128), np.float16)
    for p in range(16):
        rep16[p, p::16] = 1.0
    e36 = np.zeros((36, 36 * 128), np.float16)
    for i in range(36):
        e36[i, i * 128 : (i + 1) * 128] = 1.0
    return hky.astype(np.float32), wkx.astype(np.float32), rep16, e36


def build_program(groups=None, bn_b=B, debug=False, no_coll=False):
    nc = Bacc()
    x_in = nc.declare_dram_parameter("x", [BPC, C, P], F16, isOutput=False).ap()
    wot_in = nc.declare_dram_parameter("wot", [128, 2 * 9 * 18], F16, isOutput=False).ap()
    boff_in = nc.declare_dram_parameter("b_off", [18], F32, isOutput=False)
    wdt_in = nc.declare_dram_parameter("wdt", [128, 9 * 2 * 2 * 128], F16, isOutput=False).ap()
    wdc_in = nc.declare_dram_parameter("wdc", [128, 2 * CO * 16], F16, isOutput=False).ap()
    g1_in = nc.declare_dram_parameter("gamma1", [CO], F32, isOutput=False)
    b1_in = nc.declare_dram_parameter("beta1", [CO], F32, isOutput=False)
    g2_in = nc.declare_dram_parameter("gamma2", [CO], F32, isOutput=False)
    b2_in = nc.declare_dram_parameter("beta2", [CO], F32, isOutput=False)
    out_t = nc.declare_dram_parameter("out", [BPC, CO, PO], F16, isOutput=True).ap()
    if debug:
        off_dbg = nc.declare_dram_parameter("off_dbg", [BPC, 18, P], F32, isOutput=True).ap()
        d_dbg = nc.declare_dram_parameter("d_dbg", [BPC, CO, P], F32, isOutput=True).ap()

    hky_np, wkx_np, rep16_np, e36_np = _consts()
    hky_c = nc.inline_tensor(hky_np, "hky")
    wkx_c = nc.inline_tensor(wkx_np, "wkx")
    rep16_c = nc.inline_tensor(rep16_np, "rep16")
    e36_c = nc.inline_tensor(e36_np, "e36")

    TT = nc.vector.tensor_tensor
    TS = nc.vector.tensor_scalar
    CP = nc.vector.tensor_copy
    ACP = nc.scalar.copy

    with TileContext(nc) as tc:
        ex = tc.tile_pool
        with (
            ex(name="const", bufs=1) as cpool,
            ex(name="wts", bufs=1) as wpool,
            ex(name="big", bufs=1) as big,
            ex(name="sml", bufs=1) as sml,
            ex(name="dram", bufs=2, space="DRAM") as dpool,
        ):
            # ---------------- constants ----------------
            id16 = cpool.tile([128, 128], F16)
            masks.make_identity(nc, id16[:])
            id32 = cpool.tile([128, 128], F32)
            masks.make_identity(nc, id32[:])
            hky = cpool.tile([128, 8, 9], F32)
            nc.sync.dma_start(hky[:], hky_c[:])
            wkx = cpool.tile([128, 8, 9], F32)
            nc.sync.dma_start(wkx[:], wkx_c[:])
            rep16 = cpool.tile([16, 128], F16)
            nc.sync.dma_start(rep16[:], rep16_c[:])
            e36 = cpool.tile([36, 36 * 128], F16)
            nc.sync.dma_start(e36[:], e36_c[:])
            ones512 = cpool.tile([1, 512], F16)
            nc.vector.memset(ones512[:], 1.0)

            # ---------------- weights (pre-transposed on host) ----------------
            wot = wpool.tile([128, 2, 9, 18], F16)
            nc.sync.dma_start(wot[:], wot_in[:, :])
            wdt = wpool.tile([128, 9, 2, 2, 128], F16)
            nc.sync.dma_start(wdt[:], wdt_in[:, :])
            wdc = wpool.tile([128, 2, CO * 16], F16)
            nc.sync.dma_start(wdc[:], wdc_in[:, :])
            bof32 = sml.tile([1, 18], F32)
            nc.sync.dma_start(bof32[:], dview(boff_in.ap(), 0, [[18, 1], [1, 18]]))
            bof = wpool.tile([1, 18], F16)
            CP(bof[:], bof32[:])

            def load_cvec(t_in, name):
                t = wpool.tile([128, 2], F32, tag=name, name=name)
                nc.sync.dma_start(t[:], dview(t_in.ap(), 0, [[1, 128], [128, 2]]))
                return t

            g1 = load_cvec(g1_in, "g1")
            b1 = load_cvec(b1_in, "b1")
            g2 = load_cvec(g2_in, "g2")
            b2 = load_cvec(b2_in, "b2")

            # persistent buffers
            d_sb = big.tile([128, BPC, 2, P], F16)
            bn1l = sml.tile([128, 4], F32)
            bn2l = sml.tile([128, 4], F32)
            nc.vector.memset(bn1l[:], 0.0)
            nc.vector.memset(bn2l[:], 0.0)
            sq16 = big.tile([128, 4 * P], F16)

            xt_d = [dpool.tile([1025 * 256], F16, tag="xt", name=f"xt{b}")
                    for b in range(BPC)]

            # ======== phases 1+2 share the pipe pool (freed afterwards) ======
            with ex(name="pipe", bufs=2) as pipe:
                idx128 = [pipe.tile([128, 9, 128], I16, tag="idx128", name=f"idx{b}")
                          for b in range(BPC)]
                wall36 = [pipe.tile([36, 8 * 128], F16, tag="wall36", name=f"wall{b}")
                          for b in range(BPC)]

                # phase-2 pools open first: distinct bump-alloc region
                with contextlib.ExitStack() as stk:
                    ec = stk.enter_context
                    gtp = ec(ex(name="gtp", bufs=2))
                    comb = ec(ex(name="comb", bufs=1))
                    stp = ec(ex(name="stp", bufs=2))
                    rrs = ec(ex(name="rrs", bufs=2))
                    rrpp = ec(ex(name="rrp", bufs=1, space="PSUM"))
                    dap = ec(ex(name="dap", bufs=1, space="PSUM"))
                    xpool = ec(ex(name="xp", bufs=2))
                    padp = ec(ex(name="pads", bufs=1))
                    pp1 = ec(ex(name="pp1", bufs=1, space="PSUM"))
                    ppo = ec(ex(name="ppo", bufs=1, space="PSUM"))
                    xpad = padp.tile([128, 2, PPIX], F16)
                    nc.vector.memset(xpad[:], 0.0)
                    for b in range(BPC):
                        # ---- x load / transpose / xt store ----
                        x16 = xpool.tile([128, 2, P], F16, tag="x16", name=f"x16_{b}")
                        for cc in range(2):
                            nc.sync.dma_start(
                                apv(x16[:], cc * P, [[1, P]]),
                                x_in[b, cc * 128 : (cc + 1) * 128, :],
                            )
                        xts = xpool.tile([128, 8, 256], F16, tag="xts", name=f"xts{b}")
                        for cc in range(2):
                            for j in range(8):
                                tp = pp1.tile([128, 128], F16, tag="pp1", name="tpx")
                                nc.tensor.transpose(
                                    tp[:], apv(x16[:], cc * P + j * 128, [[1, 128]]), id16[:]
                                )
                                cp = CP if j % 2 == 0 else ACP
                                cp(xts[:, j, cc * 128 : (cc + 1) * 128], tp[:])
                        nc.sync.dma_start(
                            dview(xt_d[b], 0, [[256, 128], [128 * 256, 8], [1, 256]]),
                            xts[:],
                        )

                        # ---- offset conv ([18, P] channel-major) ----
                        for cc in range(2):
                            dst = apv(xpad[:], cc * PPIX + PADW + 1, [[PADW, 32], [1, 32]])
                            CP(dst, apv(x16[:], cc * P, [[32, 32], [1, 32]]))
                        off_cm = pipe.tile([18, P], F32, tag="off_cm", name="off_cm")
                        for half in range(2):
                            n0 = half * 512
                            ocp = ppo.tile([18, 512], F32, tag="ocp", name="ocp")
                            first = True
                            for cc in range(2):
                                for k in range(9):
                                    ky, kx = k // 3 - 1, k % 3 - 1
                                    rhs = apv(
                                        xpad[:],
                                        cc * PPIX + (1 + ky + half * 16) * PADW + 1 + kx,
                                        [[PADW, 16], [1, 32]],
                                    )
                                    nc.tensor.matmul(
                                        ocp[:],
                                        apv(wot[:], (cc * 9 + k) * 18, [[1, 18]]),
                                        rhs,
                                        start=first,
                                        stop=False,
                                    )
                                    first = False
                            nc.tensor.matmul(
                                ocp[:], bof[:], ones512[:],
                                start=False, stop=True,
                            )
                            CP(off_cm[:, n0 : n0 + 512], ocp[:])
                        if debug:
                            nc.sync.dma_start(off_dbg[b, :, :], off_cm[:])
                        off_pm = pipe.tile([128, 8, 18], F32, tag="off_pm", name="off_pm")
                        for j in range(8):
                            tp = pp1.tile([128, 18], F32, tag="pp1", name="tpo")
                            nc.tensor.transpose(
                                tp[:], off_cm[:, j * 128 : (j + 1) * 128], id32[0:18, 0:18]
                            )
                            cp = CP if j % 2 == 0 else ACP
                            cp(off_pm[:, j, :], tp[:])

                        # ---- part A: gather tokens (idx128 feeds the gathers;
                        #      emitted before the weight pipeline so phase-2
                        #      gathers can start while part B still runs) ----
                        sh = [128, 8, 9]

                        def T(tag):
                            return pipe.tile(sh, F32, tag=tag, name=tag)

                        tmp_f = T("tmp_f")
                        tmp_g = T("tmp_g")
                        tmp_i = pipe.tile(sh, I32, tag="tmp_i", name="tmp_i")

                        def floor_(dst, src):
                            TS(tmp_f[:], src, 63.5, None, ALU.add)
                            CP(tmp_i[:], tmp_f[:])
                            CP(dst, tmp_i[:])
                            TS(dst, dst, -64.0, None, ALU.add)

                        def valid(dst, src, lo, hi):
                            TS(tmp_g[:], src, lo, None, ALU.is_ge)
                            TS(dst, src, hi, None, ALU.is_le)
                            TT(dst, dst, tmp_g[:], ALU.mult)

                        dy = apv(off_pm[:], 0, [[18, 8], [2, 9]])
                        dx = apv(off_pm[:], 1, [[18, 8], [2, 9]])
                        py, px = T("py"), T("px")
                        TT(py[:], hky[:], dy, ALU.add)
                        TT(px[:], wkx[:], dx, ALU.add)
                        y0, x0 = T("y0"), T("x0")
                        floor_(y0[:], py[:])
                        floor_(x0[:], px[:])
                        yr0, yr1 = T("yr0"), T("yr1")
                        TS(yr0[:], y0[:], 0.0, 31.0, ALU.max, ALU.min)
                        TS(tmp_f[:], y0[:], 1.0, None, ALU.add)
                        TS(yr1[:], tmp_f[:], 0.0, 31.0, ALU.max, ALU.min)
                        xs = T("xs")
                        TS(xs[:], x0[:], 0.0, 30.0, ALU.max, ALU.min)
                        tok0, tok1 = T("tok0"), T("tok1")
                        TS(tmp_f[:], yr0[:], 32.0, None, ALU.mult)
                        TT(tok0[:], tmp_f[:], xs[:], ALU.add)
                        TS(tmp_f[:], yr1[:], 32.0, None, ALU.mult)
                        TT(tok1[:], tmp_f[:], xs[:], ALU.add)
                        tokf = pipe.tile([128, 2, 72], F16, tag="tokf", name="tokf")
                        CP(apv(tokf[:], 0, [[1, 72]]), tok0[:].rearrange("p a b -> p (a b)"))
                        CP(apv(tokf[:], 72, [[1, 72]]), tok1[:].rearrange("p a b -> p (a b)"))
                        tokc = pipe.tile([16, 9 * 128], F16, tag="tokc", name="tokc")
                        for r in range(2):
                            for ah in range(2):
                                pf = pp1.tile([16, 512], F32, tag="pp1", name="pf")
                                for a in range(4):
                                    nc.tensor.matmul(
                                        pf[:, a * 128 : a * 128 + 72],
                                        id16[:, (ah * 4 + a) * 16 : (ah * 4 + a + 1) * 16],
                                        apv(tokf[:], r * 72, [[1, 72]]),
                                        start=True,
                                        stop=True,
                                    )
                                src = apv(pf[:], 0, [[1, 9], [9, 8], [128, 4]])
                                dst = apv(tokc[:], r * 64 + ah * 4,
                                          [[128, 9], [8, 8], [1, 4]], nparts=16)
                                CP(dst, src)
                        for seg0, segn in ((0, 512), (512, 512), (1024, 128)):
                            idxp = pp1.tile([128, 512], F32, tag="pp1", name="idxp")
                            nc.tensor.matmul(
                                idxp[:, 0:segn],
                                rep16[:],
                                apv(tokc[:], seg0, [[1, segn]]),
                                start=True,
                                stop=True,
                            )
                            CP(apv(idx128[b][:], seg0, [[1, segn]]), idxp[:, 0:segn])

                        # ---- part B: bilinear combine weights ----
                        wy, wx = T("wy"), T("wx")
                        TT(wy[:], py[:], y0[:], ALU.subtract)
                        TT(wx[:], px[:], x0[:], ALU.subtract)
                        vy0, vy1 = T("vy0"), T("vy1")
                        valid(vy0[:], y0[:], 0.0, 31.0)
                        TS(tmp_f[:], y0[:], 1.0, None, ALU.add)
                        valid(vy1[:], tmp_f[:], 0.0, 31.0)
                        vx0, vx1 = T("vx0"), T("vx1")
                        valid(vx0[:], x0[:], 0.0, 31.0)
                        TS(tmp_f[:], x0[:], 1.0, None, ALU.add)
                        valid(vx1[:], tmp_f[:], 0.0, 31.0)
                        dl, dl2 = T("dl"), T("dl2")
                        TT(dl[:], x0[:], xs[:], ALU.subtract)
                        TT(dl2[:], dl[:], dl[:], ALU.mult)
                        i0, im, ip = T("i0"), T("im"), T("ip")
                        TS(i0[:], dl2[:], -1.0, 1.0, ALU.mult, ALU.add)
                        TT(im[:], dl2[:], dl[:], ALU.subtract)
                        TS(im[:], im[:], 0.5, None, ALU.mult)
                        TT(ip[:], dl2[:], dl[:], ALU.add)
                        TS(ip[:], ip[:], 0.5, None, ALU.mult)
                        w0, w1 = T("w0"), T("w1")
                        TS(tmp_f[:], wx[:], -1.0, 1.0, ALU.mult, ALU.add)
                        TT(w0[:], tmp_f[:], vx0[:], ALU.mult)
                        TT(w1[:], wx[:], vx1[:], ALU.mult)
                        ws0, ws1 = T("ws0"), T("ws1")
                        TT(ws0[:], w0[:], i0[:], ALU.mult)
                        TT(tmp_f[:], w1[:], im[:], ALU.mult)
                        TT(ws0[:], ws0[:], tmp_f[:], ALU.add)
                        TT(ws1[:], w1[:], i0[:], ALU.mult)
                        TT(tmp_f[:], w0[:], ip[:], ALU.mult)
                        TT(ws1[:], ws1[:], tmp_f[:], ALU.add)
                        a0, a1w = T("a0"), T("a1w")
                        TS(tmp_f[:], wy[:], -1.0, 1.0, ALU.mult, ALU.add)
                        TT(a0[:], tmp_f[:], vy0[:], ALU.mult)
                        TT(a1w[:], wy[:], vy1[:], ALU.mult)

                        # wall_pm[p, j, 4k + ci], ci = 2r + xoff
                        wall_pm = pipe.tile([128, 8, 36], F32, tag="wall_pm", name="wall_pm")
                        for ci, (rw, sl) in enumerate(
                            [(a0, ws0), (a0, ws1), (a1w, ws0), (a1w, ws1)]
                        ):
                            dst = apv(wall_pm[:], ci, [[36, 8], [4, 9]])
                            TT(dst, rw[:], sl[:], ALU.mult)
                        for j in range(8):
                            tpw = pp1.tile([36, 128], F32, tag="pp1", name="tpw")
                            nc.tensor.transpose(tpw[:], wall_pm[:, j, :], id32[:])
                            cp = CP if j % 2 == 0 else ACP
                            cp(wall36[b][:, j * 128 : (j + 1) * 128], tpw[:])

                    # ================= phase 2: deform =================
                    for b in range(BPC):
                        dacc = [
                            [dap.tile([128, 512], F32, tag=f"dacc{co}{hf}", name=f"dacc{co}{hf}") for hf in range(2)]
                            for co in range(2)
                        ]
                        src_ap = dview(xt_d[b], 0, [[256, 1024], [1, 512]])
                        for k in range(9):
                            gt = gtp.tile([128, 4, 2048], F16, tag="gt", name="gt")
                            nc.gpsimd.dma_gather(
                                gt[:],
                                src_ap,
                                idx128[b][:, k, :],
                                num_idxs=2048,
                                num_idxs_reg=2048,
                                elem_size=512,
                                elem_step=256,
                                transpose=True,
                                single_packet=False,
                            )
                            # replicate combine weights: rr_sb[:, 2*xoff + r, :]
                            rr_sb = rrs.tile([128, 4, 1024], F16, tag="rr_sb", name="rr_sb")
                            for xoff in range(2):
                                for r in range(2):
                                    krs = 4 * k + 2 * r + xoff
                                    rrp = rrpp.tile([128, 1024], F32, tag="rrp", name="rrp")
                                    for hf in range(2):
                                        nc.tensor.matmul(
                                            apv(rrp[:], hf * 512, [[1, 512]]),
                                            e36[:, krs * 128 : (krs + 1) * 128],
                                            wall36[b][:, hf * 512 : (hf + 1) * 512],
                                            start=True,
                                            stop=True,
                                        )
                                    ACP(apv(rr_sb[:], (2 * xoff + r) * 1024, [[1, 1024]]),
                                        rrp[:])
                            # combine: m = gt*rr, a1 = m_x0 + m_x1, st = a1_y0 + a1_y1
                            m_all = comb.tile([128, 8192], F16, tag="m_all", name="m_all")
                            for cc in range(2):
                                TT(
                                    apv(m_all[:], cc * 1024, [[4096, 2], [2048, 2], [1, 1024]]),
                                    apv(gt[:], cc * 2048, [[4096, 2], [1024, 2], [1, 1024]]),
                                    apv(rr_sb[:], 0, [[2048, 2], [1024, 2], [1, 1024]]),
                                    ALU.mult,
                                )
                            a1t = comb.tile([128, 4096], F16, tag="a1t", name="a1t")
                            TT(
                                a1t[:],
                                apv(m_all[:], 0, [[1, 4096]]),
                                apv(m_all[:], 4096, [[1, 4096]]),
                                ALU.add,
                            )
                            st = stp.tile([128, 2048], F16, tag="st", name="st")
                            nc.gpsimd.tensor_tensor(
                                st[:],
                                apv(a1t[:], 0, [[1, 2048]]),
                                apv(a1t[:], 2048, [[1, 2048]]),
                                ALU.add,
                            )
                            for cc in range(2):
                                for co in range(2):
                                    for hf in range(2):
                                        nc.tensor.matmul(
                                            dacc[co][hf][:],
                                            apv(wdt[:], ((k * 2 + cc) * 2 + co) * 128, [[1, 128]]),
                                            apv(st[:], cc * 1024 + hf * 512, [[1, 512]]),
                                            start=(k == 0 and cc == 0),
                                            stop=(k == 8 and cc == 1),
                                        )
                        for co in range(2):
                            for hf in range(2):
                                ACP(
                                    d_sb[:, b, co, hf * 512 : (hf + 1) * 512],
                                    dacc[co][hf][:],
                                )
                            if debug:
                                nc.gpsimd.dma_start(d_dbg[b, co * 128 : (co + 1) * 128, :], d_sb[:, b, co, :])
                            part = sml.tile([128, 1], F32, tag="part", name="part")
                            nc.scalar.activation(
                                apv(sq16[:], 0, [[1, P]]), d_sb[:, b, co, :],
                                ACTF.Copy, accum_out=part[:]
                            )
                            TT(bn1l[:, co : co + 1], bn1l[:, co : co + 1], part[:], ALU.add)
                            nc.scalar.activation(
                                apv(sq16[:], 0, [[1, P]]), d_sb[:, b, co, :],
                                ACTF.Square, accum_out=part[:]
                            )
                            TT(
                                bn1l[:, 2 + co : 3 + co],
                                bn1l[:, 2 + co : 3 + co],
                                part[:],
                                ALU.add,
                            )

            # ================= BN allreduce + coeffs =================
            def allreduce_stats(local_tile, tag):
                if no_coll:
                    g = sml.tile([128, 4], F32, tag=f"ars_{tag}", name=f"ars_{tag}")
                    CP(g[:], local_tile[:])
                    return g
                src_d = dpool.tile([128, 4], F32, tag=f"ari_{tag}", name=f"ari_{tag}")
                dst_d = dpool.tile([128, 4], F32, tag=f"aro_{tag}", name=f"aro_{tag}")
                nc.gpsimd.dma_start(src_d, local_tile[:])
                nc.gpsimd.collective_compute(
                    "AllReduce",
                    ALU.add,
                    replica_groups=groups or [list(range(NCORES))],
                    ins=[src_d.opt()],
                    outs=[dst_d.opt()],
                )
                g = sml.tile([128, 4], F32, tag=f"ars_{tag}", name=f"ars_{tag}")
                nc.gpsimd.dma_start(g[:], dst_d)
                return g

            def bn_coeffs(stats, gam, bet, count, tag):
                sc = sml.tile([128, 2], F32, tag=f"sc_{tag}", name=f"sc_{tag}")
                bi = sml.tile([128, 2], F32, tag=f"bi_{tag}", name=f"bi_{tag}")
                mean = sml.tile([128, 2], F32, tag=f"mean_{tag}", name=f"mean_{tag}")
                var = sml.tile([128, 2], F32, tag=f"var_{tag}", name=f"var_{tag}")
                t2 = sml.tile([128, 2], F32, tag=f"t2_{tag}", name=f"t2_{tag}")
                TS(mean[:], stats[:, 0:2], 1.0 / count, None, ALU.mult)
                TS(var[:], stats[:, 2:4], 1.0 / count, None, ALU.mult)
                TT(t2[:], mean[:], mean[:], ALU.mult)
                TT(var[:], var[:], t2[:], ALU.subtract)
                TS(var[:], var[:], EPS, None, ALU.add)
                nc.scalar.activation(var[:], var[:], ACTF.Sqrt)
                nc.vector.reciprocal(var[:], var[:])
                TT(sc[:], gam[:], var[:], ALU.mult)
                TT(t2[:], mean[:], sc[:], ALU.mult)
                TT(bi[:], bet[:], t2[:], ALU.subtract)
                return sc, bi

            bn1g = allreduce_stats(bn1l, "bn1")
            sc1, bi1 = bn_coeffs(bn1g, g1, b1, bn_b * P, "bn1")

            # ================= phase 3: convT =================
            TAPS = {0: [(1, 0), (3, -1)], 1: [(0, 1), (2, 0)]}
            with contextlib.ExitStack() as stk34:
                p34 = stk34.enter_context(ex(name="ph34", bufs=1))
                ypp = stk34.enter_context(ex(name="yp", bufs=2))
                zpp = stk34.enter_context(ex(name="zpp", bufs=4, space="PSUM"))
                z_sb = p34.tile([128, BPC, 2, 4, P], F16)
                ypads = []
                for b in range(BPC):
                    yp = ypp.tile([128, 2, PPIX], F16, tag="ypad", name=f"ypad{b}")
                    nc.vector.memset(yp[:], 0.0)
                    ypads.append(yp)
                for b in range(BPC):
                    ypad = ypads[b]
                    for cc in range(2):
                        dst = apv(ypad[:], cc * PPIX + PADW + 1, [[PADW, 32], [1, 32]])
                        nc.scalar.activation(
                            dst,
                            d_sb[:, b, cc, :].rearrange("p (h w) -> p h w", h=32),
                            ACTF.Silu,
                            bias=bi1[:, cc : cc + 1],
                            scale=sc1[:, cc : cc + 1],
                        )
                    for ph in range(4):
                        ry, rx = ph // 2, ph % 2
                        for co in range(2):
                            for hf in range(2):
                                zp = zpp.tile([128, 512], F32, tag="zp", name="zp")
                                first = True
                                for (kyy, dyy) in TAPS[ry]:
                                    for (kxx, dxx) in TAPS[rx]:
                                        for cc in range(2):
                                            rhs = apv(
                                                ypad[:],
                                                cc * PPIX
                                                + (1 + dyy + hf * 16) * PADW
                                                + 1
                                                + dxx,
                                                [[PADW, 16], [1, 32]],
                                            )
                                            lhsT = apv(
                                                wdc[:],
                                                cc * (CO * 16)
                                                + co * 2048
                                                + kyy * 4
                                                + kxx,
                                                [[16, 128]],
                                            )
                                            nc.tensor.matmul(
                                                zp[:],
                                                lhsT,
                                                rhs,
                                                start=first,
                                                stop=(
                                                    kyy == TAPS[ry][1][0]
                                                    and kxx == TAPS[rx][1][0]
                                                    and cc == 1
                                                ),
                                            )
                                            first = False
                                CP(
                                    z_sb[:, b, co, ph, hf * 512 : (hf + 1) * 512],
                                    zp[:],
                                )
                    for co in range(2):
                        part = sml.tile([128, 1], F32, tag="part2", name="part2")
                        nc.scalar.activation(
                            sq16[:], z_sb[:, b, co, :, :], ACTF.Copy, accum_out=part[:]
                        )
                        TT(bn2l[:, co : co + 1], bn2l[:, co : co + 1], part[:], ALU.add)
                        nc.scalar.activation(
                            sq16[:], z_sb[:, b, co, :, :], ACTF.Square, accum_out=part[:]
                        )
                        TT(
                            bn2l[:, 2 + co : 3 + co],
                            bn2l[:, 2 + co : 3 + co],
                            part[:],
                            ALU.add,
                        )

                bn2g = allreduce_stats(bn2l, "bn2")
                sc2, bi2 = bn_coeffs(bn2g, g2, b2, bn_b * PO, "bn2")

                # ============ phase 4: final BN2+SiLU + output ============
                with ex(name="outst", bufs=2) as outp:
                    for b in range(BPC):
                        for co in range(2):
                            ost = outp.tile([128, PO], F16, tag="ost", name="ost")
                            for ph in range(4):
                                ry, rx = ph // 2, ph % 2
                                dst = apv(ost[:], ry * 64 + rx, [[128, 32], [2, 32]])
                                nc.scalar.activation(
                                    dst,
                                    z_sb[:, b, co, ph, :].rearrange("p (h w) -> p h w", h=32),
                                    ACTF.Silu,
                                    bias=bi2[:, co : co + 1],
                                    scale=sc2[:, co : co + 1],
                                )
                            nc.sync.dma_start(out_t[b, co * 128 : (co + 1) * 128, :], ost[:])


    nc.finalize()
    return nc


_NC_CACHE = {}


def input_map(x_shard, inputs):
    """Per-core input dict given this core's x shard [BPC, C, P]."""
    w_off = np.asarray(inputs["w_off"], np.float32).reshape(18, 2, 128, 9)
    wot = np.ascontiguousarray(
        w_off.transpose(2, 1, 3, 0), np.float16
    )  # [ci, cc, k, o]
    w_def = np.asarray(inputs["w_def"], np.float32).reshape(2, 128, 2, 128, 9)
    wdt = np.ascontiguousarray(
        w_def.transpose(3, 4, 2, 0, 1), np.float16
    )  # [ci, k, cc, co, cl]
    w_dc = np.asarray(inputs["w_dc"], np.float32).reshape(2, 128, CO * 16)
    wdc = np.ascontiguousarray(w_dc.transpose(1, 0, 2), np.float16)  # [ci, cc, :]
    return {
        "x": np.ascontiguousarray(x_shard, np.float16),
        "wot": wot.reshape(128, 2 * 9 * 18),
        "b_off": np.ascontiguousarray(inputs["b_off"], np.float32),
        "wdt": wdt.reshape(128, 9 * 2 * 2 * 128),
        "wdc": wdc.reshape(128, 2 * CO * 16),
        "gamma1": np.ascontiguousarray(inputs["gamma1"], np.float32),
        "beta1": np.ascontiguousarray(inputs["beta1"], np.float32),
        "gamma2": np.ascontiguousarray(inputs["gamma2"], np.float32),
        "beta2": np.ascontiguousarray(inputs["beta2"], np.float32),
    }


def kernel(**inputs):
    if "nc" not in _NC_CACHE:
        _NC_CACHE["nc"] = build_program()
    nc = _NC_CACHE["nc"]

    xr = np.ascontiguousarray(inputs["x"], dtype=np.float32).reshape(B, C, P)
    in_maps = [
        input_map(xr[core * BPC : (core + 1) * BPC], inputs)
        for core in range(NCORES)
    ]
    res = run_bass_kernel_spmd(nc, in_maps, list(range(NCORES)))
    out = np.concatenate([res.results[i]["out"] for i in range(NCORES)], axis=0)
    return out.reshape(B, CO, HO, WO).astype(np.float32)


# revision 32
# speedup vs baseline: 1.0187x; 1.0187x over previous
"""Trainium2 Bass kernel for nn_DeconvBlock (offset conv -> deformable conv
-> BN+SiLU -> ConvTranspose2d(4,2,1) -> BN+SiLU), data-parallel over batch
on 8 NeuronCores with sync-BN allreduces.

v3: phased schedule (prep -> deform -> AR1 -> convT -> AR2 -> out) with the
gather-token pipeline hoisted ahead of the bilinear-weight pipeline so the
gathers overlap prep of the next batch element; prep PSUM evacuations split
across DVE/ACT; host-side pre-transposed f16 weights; wide 3-op DVE combine
plus one GPSIMD add per tap; f16 I/O; one-time pad-border memsets.
"""
import contextlib
import sys

sys.path.insert(0, "/opt/trn_rl_repo")

import numpy as np

import concourse.bass as bass
import concourse.mybir as mybir
from concourse import masks
from concourse.bacc import Bacc
from concourse.bass_types import AP
from concourse.bass_utils import run_bass_kernel_spmd
from concourse.tile import TileContext
from concourse.vector_clock import ScopedClock

# ---------------------------------------------------------------------------
# This walrus build rejects >1 sync-wait command on a Drain: split the
# TileContext tail-drain waits into a chain of single-wait drains.
from concourse import tile as _tile


def _patched_drain_and_barrier(self, tick_clock, wait_clock):
    nc = self.nc
    gc = tick_clock.global_clock
    procs = [(None, proc, tick) for proc, tick in enumerate(gc) if tick > 0]
    for scope, proc, tick in procs:
        sc = ScopedClock()
        sc.require_at_least(scope, proc, tick)
        d = nc.sync.drain()
        wait_clock.add_sem_waits(d.ins, sc)
    if not procs:
        nc.sync.drain()
    nc.all_engine_barrier()
    assert self.sems is not None
    popped = nc._tile_sem_poison_stack.pop()
    assert popped is self._sem_poison
    nc.clear_and_free_semaphores(list(self.sems.allocated().values()))
    nc.all_engine_barrier()


_tile.TileContext._drain_and_barrier = _patched_drain_and_barrier
# ---------------------------------------------------------------------------

F32 = mybir.dt.float32
F16 = mybir.dt.float16
I16 = mybir.dt.int16
I32 = mybir.dt.int32
ALU = mybir.AluOpType
ACTF = mybir.ActivationFunctionType

B, C, CO, H, W = 16, 256, 256, 32, 32
P = H * W  # 1024
NCORES = 8
BPC = B // NCORES  # batch elems per core
HO, WO = 2 * H, 2 * W
PO = HO * WO  # 4096
EPS = 1e-5
PADW = 34  # padded image row stride (34x34)
PPIX = PADW * PADW  # 1156


def apv(base, off, dims, nparts=None):
    """Free-dim view of an SBUF AP: keep its partition dim (stride), replace
    free dims. dims entries are [step, count] in elements of the tile row."""
    p = [base.ap[0][0], nparts if nparts is not None else base.ap[0][1]]
    return AP(tensor=base.tensor, offset=base.offset + off, ap=[p] + [list(d) for d in dims])


def dview(base, off, dims):
    """Arbitrary flat view of a DRAM AP."""
    return AP(tensor=base.tensor, offset=base.offset + off, ap=[list(d) for d in dims])


def _consts():
    q = np.arange(128)
    j = np.arange(8)
    k = np.arange(9)
    ky = k // 3 - 1
    kx = k % 3 - 1
    h = 4 * j[None, :, None] + (q[:, None, None] // 32)
    w = (q % 32)[:, None, None] + np.zeros((1, 8, 1))
    hky = (h + ky[None, None, :]) + np.zeros((128, 8, 9))
    wkx = (w + kx[None, None, :]) + np.zeros((128, 8, 9))
    rep16 = np.zeros((16, 128), np.float16)
    for p in range(16):
        rep16[p, p::16] = 1.0
    e36 = np.zeros((36, 36 * 128), np.float16)
    for i in range(36):
        e36[i, i * 128 : (i + 1) * 128] = 1.0
    return hky.astype(np.float32), wkx.astype(np.float32), rep16, e36


def build_program(groups=None, bn_b=B, debug=False, no_coll=False):
    nc = Bacc()
    x_in = nc.declare_dram_parameter("x", [BPC, C, P], F16, isOutput=False).ap()
    wot_in = nc.declare_dram_parameter("wot", [128, 2 * 9 * 18], F16, isOutput=False).ap()
    boff_in = nc.declare_dram_parameter("b_off", [18], F32, isOutput=False)
    wdt_in = nc.declare_dram_parameter("wdt", [128, 9 * 2 * 2 * 128], F16, isOutput=False).ap()
    wdc_in = nc.declare_dram_parameter("wdc", [128, 2 * CO * 16], F16, isOutput=False).ap()
    g1_in = nc.declare_dram_parameter("gamma1", [CO], F32, isOutput=False)
    b1_in = nc.declare_dram_parameter("beta1", [CO], F32, isOutput=False)
    g2_in = nc.declare_dram_parameter("gamma2", [CO], F32, isOutput=False)
    b2_in = nc.declare_dram_parameter("beta2", [CO], F32, isOutput=False)
    out_t = nc.declare_dram_parameter("out", [BPC, CO, PO], F16, isOutput=True).ap()
    if debug:
        off_dbg = nc.declare_dram_parameter("off_dbg", [BPC, 18, P], F32, isOutput=True).ap()
        d_dbg = nc.declare_dram_parameter("d_dbg", [BPC, CO, P], F32, isOutput=True).ap()

    hky_np, wkx_np, rep16_np, e36_np = _consts()
    hky_c = nc.inline_tensor(hky_np, "hky")
    wkx_c = nc.inline_tensor(wkx_np, "wkx")
    rep16_c = nc.inline_tensor(rep16_np, "rep16")
    e36_c = nc.inline_tensor(e36_np, "e36")

    TT = nc.vector.tensor_tensor
    TS = nc.vector.tensor_scalar
    CP = nc.vector.tensor_copy
    ACP = nc.scalar.copy

    with TileContext(nc) as tc:
        ex = tc.tile_pool
        with (
            ex(name="const", bufs=1) as cpool,
            ex(name="wts", bufs=1) as wpool,
            ex(name="big", bufs=1) as big,
            ex(name="sml", bufs=1) as sml,
            ex(name="dram", bufs=2, space="DRAM") as dpool,
        ):
            # ---------------- constants ----------------
            id16 = cpool.tile([128, 128], F16)
            masks.make_identity(nc, id16[:])
            id32 = cpool.tile([128, 128], F32)
            masks.make_identity(nc, id32[:])
            hky = cpool.tile([128, 8, 9], F32)
            nc.sync.dma_start(hky[:], hky_c[:])
            wkx = cpool.tile([128, 8, 9], F32)
            nc.sync.dma_start(wkx[:], wkx_c[:])
            rep16 = cpool.tile([16, 128], F16)
            nc.sync.dma_start(rep16[:], rep16_c[:])
            e36 = cpool.tile([36, 36 * 128], F16)
            nc.sync.dma_start(e36[:], e36_c[:])
            ones512 = cpool.tile([1, 512], F16)
            nc.vector.memset(ones512[:], 1.0)

            # ---------------- weights (pre-transposed on host) ----------------
            wot = wpool.tile([128, 2, 9, 18], F16)
            nc.sync.dma_start(wot[:], wot_in[:, :])
            wdt = wpool.tile([128, 9, 2, 2, 128], F16)
            wdc = wpool.tile([128, 2, CO * 16], F16)
            bof32 = sml.tile([1, 18], F32)
            nc.sync.dma_start(bof32[:], dview(boff_in.ap(), 0, [[18, 1], [1, 18]]))
            bof = wpool.tile([1, 18], F16)
            CP(bof[:], bof32[:])

            def load_cvec(t_in, name):
                t = wpool.tile([128, 2], F32, tag=name, name=name)
                nc.sync.dma_start(t[:], dview(t_in.ap(), 0, [[1, 128], [128, 2]]))
                return t

            g1 = load_cvec(g1_in, "g1")
            b1 = load_cvec(b1_in, "b1")
            g2 = load_cvec(g2_in, "g2")
            b2 = load_cvec(b2_in, "b2")

            # persistent buffers
            d_sb = big.tile([128, BPC, 2, P], F16)
            bn1l = sml.tile([128, 4], F32)
            bn2l = sml.tile([128, 4], F32)
            nc.vector.memset(bn1l[:], 0.0)
            nc.vector.memset(bn2l[:], 0.0)
            sq16 = big.tile([128, 4 * P], F16)

            xt_d = [dpool.tile([1025 * 256], F16, tag="xt", name=f"xt{b}")
                    for b in range(BPC)]

            # ======== phases 1+2 share the pipe pool (freed afterwards) ======
            with ex(name="pipe", bufs=2) as pipe:
                idx128 = [pipe.tile([128, 9, 128], I16, tag="idx128", name=f"idx{b}")
                          for b in range(BPC)]
                wall36 = [pipe.tile([36, 8 * 128], F16, tag="wall36", name=f"wall{b}")
                          for b in range(BPC)]

                # phase-2 pools open first: distinct bump-alloc region
                with contextlib.ExitStack() as stk:
                    ec = stk.enter_context
                    gtp = ec(ex(name="gtp", bufs=2))
                    comb = ec(ex(name="comb", bufs=1))
                    stp = ec(ex(name="stp", bufs=2))
                    rrs = ec(ex(name="rrs", bufs=2))
                    rrpp = ec(ex(name="rrp", bufs=1, space="PSUM"))
                    dap = ec(ex(name="dap", bufs=1, space="PSUM"))
                    xpool = ec(ex(name="xp", bufs=2))
                    padp = ec(ex(name="pads", bufs=1))
                    pp1 = ec(ex(name="pp1", bufs=1, space="PSUM"))
                    ppo = ec(ex(name="ppo", bufs=1, space="PSUM"))
                    xpad = padp.tile([128, 2, PPIX], F16)
                    nc.vector.memset(xpad[:], 0.0)
                    for b in range(BPC):
                        # ---- x load / transpose / xt store ----
                        x16 = xpool.tile([128, 2, P], F16, tag="x16", name=f"x16_{b}")
                        for cc in range(2):
                            nc.sync.dma_start(
                                apv(x16[:], cc * P, [[1, P]]),
                                x_in[b, cc * 128 : (cc + 1) * 128, :],
                            )
                        xts = xpool.tile([128, 8, 256], F16, tag="xts", name=f"xts{b}")
                        for cc in range(2):
                            for j in range(8):
                                tp = pp1.tile([128, 128], F16, tag="pp1", name="tpx")
                                nc.tensor.transpose(
                                    tp[:], apv(x16[:], cc * P + j * 128, [[1, 128]]), id16[:]
                                )
                                cp = CP if j % 2 == 0 else ACP
                                cp(xts[:, j, cc * 128 : (cc + 1) * 128], tp[:])
                        nc.sync.dma_start(
                            dview(xt_d[b], 0, [[256, 128], [128 * 256, 8], [1, 256]]),
                            xts[:],
                        )

                        # ---- offset conv ([18, P] channel-major) ----
                        for cc in range(2):
                            dst = apv(xpad[:], cc * PPIX + PADW + 1, [[PADW, 32], [1, 32]])
                            CP(dst, apv(x16[:], cc * P, [[32, 32], [1, 32]]))
                        off_cm = pipe.tile([18, P], F32, tag="off_cm", name="off_cm")
                        for half in range(2):
                            n0 = half * 512
                            ocp = ppo.tile([18, 512], F32, tag="ocp", name="ocp")
                            first = True
                            for cc in range(2):
                                for k in range(9):
                                    ky, kx = k // 3 - 1, k % 3 - 1
                                    rhs = apv(
                                        xpad[:],
                                        cc * PPIX + (1 + ky + half * 16) * PADW + 1 + kx,
                                        [[PADW, 16], [1, 32]],
                                    )
                                    nc.tensor.matmul(
                                        ocp[:],
                                        apv(wot[:], (cc * 9 + k) * 18, [[1, 18]]),
                                        rhs,
                                        start=first,
                                        stop=False,
                                    )
                                    first = False
                            nc.tensor.matmul(
                                ocp[:], bof[:], ones512[:],
                                start=False, stop=True,
                            )
                            CP(off_cm[:, n0 : n0 + 512], ocp[:])
                        if debug:
                            nc.sync.dma_start(off_dbg[b, :, :], off_cm[:])
                        off_pm = pipe.tile([128, 8, 18], F32, tag="off_pm", name="off_pm")
                        for j in range(8):
                            tp = pp1.tile([128, 18], F32, tag="pp1", name="tpo")
                            nc.tensor.transpose(
                                tp[:], off_cm[:, j * 128 : (j + 1) * 128], id32[0:18, 0:18]
                            )
                            cp = CP if j % 2 == 0 else ACP
                            cp(off_pm[:, j, :], tp[:])

                        # ---- part A: gather tokens (idx128 feeds the gathers;
                        #      emitted before the weight pipeline so phase-2
                        #      gathers can start while part B still runs) ----
                        sh = [128, 8, 9]

                        def T(tag):
                            return pipe.tile(sh, F32, tag=tag, name=tag)

                        tmp_f = T("tmp_f")
                        tmp_g = T("tmp_g")
                        tmp_i = pipe.tile(sh, I32, tag="tmp_i", name="tmp_i")

                        def floor_(dst, src):
                            TS(tmp_f[:], src, 63.5, None, ALU.add)
                            CP(tmp_i[:], tmp_f[:])
                            CP(dst, tmp_i[:])
                            TS(dst, dst, -64.0, None, ALU.add)

                        def valid(dst, src, lo, hi):
                            TS(tmp_g[:], src, lo, None, ALU.is_ge)
                            TS(dst, src, hi, None, ALU.is_le)
                            TT(dst, dst, tmp_g[:], ALU.mult)

                        dy = apv(off_pm[:], 0, [[18, 8], [2, 9]])
                        dx = apv(off_pm[:], 1, [[18, 8], [2, 9]])
                        py, px = T("py"), T("px")
                        TT(py[:], hky[:], dy, ALU.add)
                        TT(px[:], wkx[:], dx, ALU.add)
                        y0, x0 = T("y0"), T("x0")
                        floor_(y0[:], py[:])
                        floor_(x0[:], px[:])
                        yr0, yr1 = T("yr0"), T("yr1")
                        TS(yr0[:], y0[:], 0.0, 31.0, ALU.max, ALU.min)
                        TS(tmp_f[:], y0[:], 1.0, None, ALU.add)
                        TS(yr1[:], tmp_f[:], 0.0, 31.0, ALU.max, ALU.min)
                        xs = T("xs")
                        TS(xs[:], x0[:], 0.0, 30.0, ALU.max, ALU.min)
                        tok0, tok1 = T("tok0"), T("tok1")
                        TS(tmp_f[:], yr0[:], 32.0, None, ALU.mult)
                        TT(tok0[:], tmp_f[:], xs[:], ALU.add)
                        TS(tmp_f[:], yr1[:], 32.0, None, ALU.mult)
                        TT(tok1[:], tmp_f[:], xs[:], ALU.add)
                        tokf = pipe.tile([128, 2, 72], F16, tag="tokf", name="tokf")
                        CP(apv(tokf[:], 0, [[1, 72]]), tok0[:].rearrange("p a b -> p (a b)"))
                        CP(apv(tokf[:], 72, [[1, 72]]), tok1[:].rearrange("p a b -> p (a b)"))
                        tokc = pipe.tile([16, 9 * 128], F16, tag="tokc", name="tokc")
                        for r in range(2):
                            for ah in range(2):
                                pf = pp1.tile([16, 512], F32, tag="pp1", name="pf")
                                for a in range(4):
                                    nc.tensor.matmul(
                                        pf[:, a * 128 : a * 128 + 72],
                                        id16[:, (ah * 4 + a) * 16 : (ah * 4 + a + 1) * 16],
                                        apv(tokf[:], r * 72, [[1, 72]]),
                                        start=True,
                                        stop=True,
                                    )
                                src = apv(pf[:], 0, [[1, 9], [9, 8], [128, 4]])
                                dst = apv(tokc[:], r * 64 + ah * 4,
                                          [[128, 9], [8, 8], [1, 4]], nparts=16)
                                CP(dst, src)
                        for seg0, segn in ((0, 512), (512, 512), (1024, 128)):
                            idxp = pp1.tile([128, 512], F32, tag="pp1", name="idxp")
                            nc.tensor.matmul(
                                idxp[:, 0:segn],
                                rep16[:],
                                apv(tokc[:], seg0, [[1, segn]]),
                                start=True,
                                stop=True,
                            )
                            CP(apv(idx128[b][:], seg0, [[1, segn]]), idxp[:, 0:segn])

                        # ---- part B: bilinear combine weights ----
                        wy, wx = T("wy"), T("wx")
                        TT(wy[:], py[:], y0[:], ALU.subtract)
                        TT(wx[:], px[:], x0[:], ALU.subtract)
                        vy0, vy1 = T("vy0"), T("vy1")
                        valid(vy0[:], y0[:], 0.0, 31.0)
                        TS(tmp_f[:], y0[:], 1.0, None, ALU.add)
                        valid(vy1[:], tmp_f[:], 0.0, 31.0)
                        vx0, vx1 = T("vx0"), T("vx1")
                        valid(vx0[:], x0[:], 0.0, 31.0)
                        TS(tmp_f[:], x0[:], 1.0, None, ALU.add)
                        valid(vx1[:], tmp_f[:], 0.0, 31.0)
                        dl, dl2 = T("dl"), T("dl2")
                        TT(dl[:], x0[:], xs[:], ALU.subtract)
                        TT(dl2[:], dl[:], dl[:], ALU.mult)
                        i0, im, ip = T("i0"), T("im"), T("ip")
                        TS(i0[:], dl2[:], -1.0, 1.0, ALU.mult, ALU.add)
                        TT(im[:], dl2[:], dl[:], ALU.subtract)
                        TS(im[:], im[:], 0.5, None, ALU.mult)
                        TT(ip[:], dl2[:], dl[:], ALU.add)
                        TS(ip[:], ip[:], 0.5, None, ALU.mult)
                        w0, w1 = T("w0"), T("w1")
                        TS(tmp_f[:], wx[:], -1.0, 1.0, ALU.mult, ALU.add)
                        TT(w0[:], tmp_f[:], vx0[:], ALU.mult)
                        TT(w1[:], wx[:], vx1[:], ALU.mult)
                        ws0, ws1 = T("ws0"), T("ws1")
                        TT(ws0[:], w0[:], i0[:], ALU.mult)
                        TT(tmp_f[:], w1[:], im[:], ALU.mult)
                        TT(ws0[:], ws0[:], tmp_f[:], ALU.add)
                        TT(ws1[:], w1[:], i0[:], ALU.mult)
                        TT(tmp_f[:], w0[:], ip[:], ALU.mult)
                        TT(ws1[:], ws1[:], tmp_f[:], ALU.add)
                        a0, a1w = T("a0"), T("a1w")
                        TS(tmp_f[:], wy[:], -1.0, 1.0, ALU.mult, ALU.add)
                        TT(a0[:], tmp_f[:], vy0[:], ALU.mult)
                        TT(a1w[:], wy[:], vy1[:], ALU.mult)

                        # wall_pm[p, j, 4k + ci], ci = 2r + xoff
                        wall_pm = pipe.tile([128, 8, 36], F32, tag="wall_pm", name="wall_pm")
                        for ci, (rw, sl) in enumerate(
                            [(a0, ws0), (a0, ws1), (a1w, ws0), (a1w, ws1)]
                        ):
                            dst = apv(wall_pm[:], ci, [[36, 8], [4, 9]])
                            TT(dst, rw[:], sl[:], ALU.mult)
                        for j in range(8):
                            tpw = pp1.tile([36, 128], F32, tag="pp1", name="tpw")
                            nc.tensor.transpose(tpw[:], wall_pm[:, j, :], id32[:])
                            cp = CP if j % 2 == 0 else ACP
                            cp(wall36[b][:, j * 128 : (j + 1) * 128], tpw[:])

                    # deform/convT weights: loaded after the startup-critical
                    # x/prep DMAs (first use is ~55us / ~240us in)
                    nc.sync.dma_start(wdt[:], wdt_in[:, :])
                    nc.sync.dma_start(wdc[:], wdc_in[:, :])

                    # ================= phase 2: deform =================
                    for b in range(BPC):
                        dacc = [
                            [dap.tile([128, 512], F32, tag=f"dacc{co}{hf}", name=f"dacc{co}{hf}") for hf in range(2)]
                            for co in range(2)
                        ]
                        src_ap = dview(xt_d[b], 0, [[256, 1024], [1, 512]])
                        for k in range(9):
                            gt = gtp.tile([128, 4, 2048], F16, tag="gt", name="gt")
                            nc.gpsimd.dma_gather(
                                gt[:],
                                src_ap,
                                idx128[b][:, k, :],
                                num_idxs=2048,
                                num_idxs_reg=2048,
                                elem_size=512,
                                elem_step=256,
                                transpose=True,
                                single_packet=False,
                            )
                            # replicate combine weights: rr_sb[:, 2*xoff + r, :]
                            rr_sb = rrs.tile([128, 4, 1024], F16, tag="rr_sb", name="rr_sb")
                            for xoff in range(2):
                                for r in range(2):
                                    krs = 4 * k + 2 * r + xoff
                                    rrp = rrpp.tile([128, 1024], F32, tag="rrp", name="rrp")
                                    for hf in range(2):
                                        nc.tensor.matmul(
                                            apv(rrp[:], hf * 512, [[1, 512]]),
                                            e36[:, krs * 128 : (krs + 1) * 128],
                                            wall36[b][:, hf * 512 : (hf + 1) * 512],
                                            start=True,
                                            stop=True,
                                        )
                                    ACP(apv(rr_sb[:], (2 * xoff + r) * 1024, [[1, 1024]]),
                                        rrp[:])
                            # combine: m = gt*rr, a1 = m_x0 + m_x1, st = a1_y0 + a1_y1
                            m_all = comb.tile([128, 8192], F16, tag="m_all", name="m_all")
                            for cc in range(2):
                                TT(
                                    apv(m_all[:], cc * 1024, [[4096, 2], [2048, 2], [1, 1024]]),
                                    apv(gt[:], cc * 2048, [[4096, 2], [1024, 2], [1, 1024]]),
                                    apv(rr_sb[:], 0, [[2048, 2], [1024, 2], [1, 1024]]),
                                    ALU.mult,
                                )
                            a1t = comb.tile([128, 4096], F16, tag="a1t", name="a1t")
                            TT(
                                a1t[:],
                                apv(m_all[:], 0, [[1, 4096]]),
                                apv(m_all[:], 4096, [[1, 4096]]),
                                ALU.add,
                            )
                            st = stp.tile([128, 2048], F16, tag="st", name="st")
                            nc.gpsimd.tensor_tensor(
                                st[:],
                                apv(a1t[:], 0, [[1, 2048]]),
                                apv(a1t[:], 2048, [[1, 2048]]),
                                ALU.add,
                            )
                            for cc in range(2):
                                for co in range(2):
                                    for hf in range(2):
                                        nc.tensor.matmul(
                                            dacc[co][hf][:],
                                            apv(wdt[:], ((k * 2 + cc) * 2 + co) * 128, [[1, 128]]),
                                            apv(st[:], cc * 1024 + hf * 512, [[1, 512]]),
                                            start=(k == 0 and cc == 0),
                                            stop=(k == 8 and cc == 1),
                                        )
                        for co in range(2):
                            for hf in range(2):
                                ACP(
                                    d_sb[:, b, co, hf * 512 : (hf + 1) * 512],
                                    dacc[co][hf][:],
                                )
                            if debug:
                                nc.gpsimd.dma_start(d_dbg[b, co * 128 : (co + 1) * 128, :], d_sb[:, b, co, :])
                            part = sml.tile([128, 1], F32, tag="part", name="part")
                            nc.scalar.activation(
                                apv(sq16[:], 0, [[1, P]]), d_sb[:, b, co, :],
                                ACTF.Copy, accum_out=part[:]
                            )
                            TT(bn1l[:, co : co + 1], bn1l[:, co : co + 1], part[:], ALU.add)
                            nc.scalar.activation(
                                apv(sq16[:], 0, [[1, P]]), d_sb[:, b, co, :],
                                ACTF.Square, accum_out=part[:]
                            )
                            TT(
                                bn1l[:, 2 + co : 3 + co],
                                bn1l[:, 2 + co : 3 + co],
                                part[:],
                                ALU.add,
                            )

            # ================= BN allreduce + coeffs =================
            def allreduce_stats(local_tile, tag):
                if no_coll:
                    g = sml.tile([128, 4], F32, tag=f"ars_{tag}", name=f"ars_{tag}")
                    CP(g[:], local_tile[:])
                    return g
                src_d = dpool.tile([128, 4], F32, tag=f"ari_{tag}", name=f"ari_{tag}")
                dst_d = dpool.tile([128, 4], F32, tag=f"aro_{tag}", name=f"aro_{tag}")
                nc.gpsimd.dma_start(src_d, local_tile[:])
                nc.gpsimd.collective_compute(
                    "AllReduce",
                    ALU.add,
                    replica_groups=groups or [list(range(NCORES))],
                    ins=[src_d.opt()],
                    outs=[dst_d.opt()],
                )
                g = sml.tile([128, 4], F32, tag=f"ars_{tag}", name=f"ars_{tag}")
                nc.gpsimd.dma_start(g[:], dst_d)
                return g

            def bn_coeffs(stats, gam, bet, count, tag):
                sc = sml.tile([128, 2], F32, tag=f"sc_{tag}", name=f"sc_{tag}")
                bi = sml.tile([128, 2], F32, tag=f"bi_{tag}", name=f"bi_{tag}")
                mean = sml.tile([128, 2], F32, tag=f"mean_{tag}", name=f"mean_{tag}")
                var = sml.tile([128, 2], F32, tag=f"var_{tag}", name=f"var_{tag}")
                t2 = sml.tile([128, 2], F32, tag=f"t2_{tag}", name=f"t2_{tag}")
                TS(mean[:], stats[:, 0:2], 1.0 / count, None, ALU.mult)
                TS(var[:], stats[:, 2:4], 1.0 / count, None, ALU.mult)
                TT(t2[:], mean[:], mean[:], ALU.mult)
                TT(var[:], var[:], t2[:], ALU.subtract)
                TS(var[:], var[:], EPS, None, ALU.add)
                nc.scalar.activation(var[:], var[:], ACTF.Sqrt)
                nc.vector.reciprocal(var[:], var[:])
                TT(sc[:], gam[:], var[:], ALU.mult)
                TT(t2[:], mean[:], sc[:], ALU.mult)
                TT(bi[:], bet[:], t2[:], ALU.subtract)
                return sc, bi

            bn1g = allreduce_stats(bn1l, "bn1")
            sc1, bi1 = bn_coeffs(bn1g, g1, b1, bn_b * P, "bn1")

            # ================= phase 3: convT =================
            TAPS = {0: [(1, 0), (3, -1)], 1: [(0, 1), (2, 0)]}
            with contextlib.ExitStack() as stk34:
                p34 = stk34.enter_context(ex(name="ph34", bufs=1))
                ypp = stk34.enter_context(ex(name="yp", bufs=2))
                zpp = stk34.enter_context(ex(name="zpp", bufs=4, space="PSUM"))
                z_sb = p34.tile([128, BPC, 2, 4, P], F16)
                ypads = []
                for b in range(BPC):
                    yp = ypp.tile([128, 2, PPIX], F16, tag="ypad", name=f"ypad{b}")
                    nc.vector.memset(yp[:], 0.0)
                    ypads.append(yp)
                for b in range(BPC):
                    ypad = ypads[b]
                    for cc in range(2):
                        dst = apv(ypad[:], cc * PPIX + PADW + 1, [[PADW, 32], [1, 32]])
                        nc.scalar.activation(
                            dst,
                            d_sb[:, b, cc, :].rearrange("p (h w) -> p h w", h=32),
                            ACTF.Silu,
                            bias=bi1[:, cc : cc + 1],
                            scale=sc1[:, cc : cc + 1],
                        )
                    for ph in range(4):
                        ry, rx = ph // 2, ph % 2
                        for co in range(2):
                            for hf in range(2):
                                zp = zpp.tile([128, 512], F32, tag="zp", name="zp")
                                first = True
                                for (kyy, dyy) in TAPS[ry]:
                                    for (kxx, dxx) in TAPS[rx]:
                                        for cc in range(2):
                                            rhs = apv(
                                                ypad[:],
                                                cc * PPIX
                                                + (1 + dyy + hf * 16) * PADW
                                                + 1
                                                + dxx,
                                                [[PADW, 16], [1, 32]],
                                            )
                                            lhsT = apv(
                                                wdc[:],
                                                cc * (CO * 16)
                                                + co * 2048
                                                + kyy * 4
                                                + kxx,
                                                [[16, 128]],
                                            )
                                            nc.tensor.matmul(
                                                zp[:],
                                                lhsT,
                                                rhs,
                                                start=first,
                                                stop=(
                                                    kyy == TAPS[ry][1][0]
                                                    and kxx == TAPS[rx][1][0]
                                                    and cc == 1
                                                ),
                                            )
                                            first = False
                                CP(
                                    z_sb[:, b, co, ph, hf * 512 : (hf + 1) * 512],
                                    zp[:],
                                )
                    for co in range(2):
                        part = sml.tile([128, 1], F32, tag="part2", name="part2")
                        nc.scalar.activation(
                            sq16[:], z_sb[:, b, co, :, :], ACTF.Copy, accum_out=part[:]
                        )
                        TT(bn2l[:, co : co + 1], bn2l[:, co : co + 1], part[:], ALU.add)
                        nc.scalar.activation(
                            sq16[:], z_sb[:, b, co, :, :], ACTF.Square, accum_out=part[:]
                        )
                        TT(
                            bn2l[:, 2 + co : 3 + co],
                            bn2l[:, 2 + co : 3 + co],
                            part[:],
                            ALU.add,
                        )

                bn2g = allreduce_stats(bn2l, "bn2")
                sc2, bi2 = bn_coeffs(bn2g, g2, b2, bn_b * PO, "bn2")

                # ============ phase 4: final BN2+SiLU + output ============
                with ex(name="outst", bufs=2) as outp:
                    for b in range(BPC):
                        for co in range(2):
                            ost = outp.tile([128, PO], F16, tag="ost", name="ost")
                            for ph in range(4):
                                ry, rx = ph // 2, ph % 2
                                dst = apv(ost[:], ry * 64 + rx, [[128, 32], [2, 32]])
                                nc.scalar.activation(
                                    dst,
                                    z_sb[:, b, co, ph, :].rearrange("p (h w) -> p h w", h=32),
                                    ACTF.Silu,
                                    bias=bi2[:, co : co + 1],
                                    scale=sc2[:, co : co + 1],
                                )
                            nc.sync.dma_start(out_t[b, co * 128 : (co + 1) * 128, :], ost[:])


    nc.finalize()
    return nc


_NC_CACHE = {}


def input_map(x_shard, inputs):
    """Per-core input dict given this core's x shard [BPC, C, P]."""
    w_off = np.asarray(inputs["w_off"], np.float32).reshape(18, 2, 128, 9)
    wot = np.ascontiguousarray(
        w_off.transpose(2, 1, 3, 0), np.float16
    )  # [ci, cc, k, o]
    w_def = np.asarray(inputs["w_def"], np.float32).reshape(2, 128, 2, 128, 9)
    wdt = np.ascontiguousarray(
        w_def.transpose(3, 4, 2, 0, 1), np.float16
    )  # [ci, k, cc, co, cl]
    w_dc = np.asarray(inputs["w_dc"], np.float32).reshape(2, 128, CO * 16)
    wdc = np.ascontiguousarray(w_dc.transpose(1, 0, 2), np.float16)  # [ci, cc, :]
    return {
        "x": np.ascontiguousarray(x_shard, np.float16),
        "wot": wot.reshape(128, 2 * 9 * 18),
        "b_off": np.ascontiguousarray(inputs["b_off"], np.float32),
        "wdt": wdt.reshape(128, 9 * 2 * 2 * 128),
        "wdc": wdc.reshape(128, 2 * CO * 16),
        "gamma1": np.ascontiguousarray(inputs["gamma1"], np.float32),
        "beta1": np.ascontiguousarray(inputs["beta1"], np.float32),
        "gamma2": np.ascontiguousarray(inputs["gamma2"], np.float32),
        "beta2": np.ascontiguousarray(inputs["beta2"], np.float32),
    }


def kernel(**inputs):
    if "nc" not in _NC_CACHE:
        _NC_CACHE["nc"] = build_program()
    nc = _NC_CACHE["nc"]

    xr = np.ascontiguousarray(inputs["x"], dtype=np.float32).reshape(B, C, P)
    in_maps = [
        input_map(xr[core * BPC : (core + 1) * BPC], inputs)
        for core in range(NCORES)
    ]
    res = run_bass_kernel_spmd(nc, in_maps, list(range(NCORES)))
    out = np.concatenate([res.results[i]["out"] for i in range(NCORES)], axis=0)
    return out.reshape(B, CO, HO, WO).astype(np.float32)
